# revision 10
# baseline (speedup 1.0000x reference)
"""Trainium2 Bass kernel for nn_PolicyNetwork (ragged gather + MLP policy head).

Strategy (8 NeuronCores, batch-sharded, no collectives):
  - Each core owns 32 of the 256 batch rows.
  - Embedding tables (rel/ent/trip) are replicated in each core's DRAM.
  - MLP X2 = relu(obs@W1+b1)@W2+b2 computed on PE (biases folded in via
    an appended ones-row on the activations / extra weight row).
  - Action embeddings are row-gathered straight from DRAM with batched
    indirect DMA (one descriptor per 800B row), landing interleaved as
    [128 queries, 600] tiles (rel|ent|trip concatenated).
  - logits[b,a] = <emb row, X2[b]> via fused multiply+reduce
    (tensor_tensor_reduce on DVE / scalar_tensor_tensor on GPSIMD).
  - mask, softmax and entropy computed on-chip; outputs per core are
    dist [32,512] and ent [32,1]; host just concatenates.
"""

import numpy as np

B, A, D = 256, 512, 200
AD = 3 * D
N_REL, N_ENT, N_TRIP = 500, 100000, 200000
NCORES = 8
BL = B // NCORES            # 32 local batch rows per core
GROUPS = 8                  # gather groups per core
BPG = BL // GROUPS          # 4 batch rows per group
SLOTS = BPG * 4             # 16 query slots per group tile ([128,16,600])
HUGE = 1e31
EPS = float(np.finfo(np.float64).eps)
# fraction of dot products sent to GPSIMD: every Nth tile (0 disables).
# NOTE: walrus rejects TensorScalarPtr on Pool (NCC_IXCG966), keep 0.
GPSIMD_EVERY = 0

_CACHE = {}


def _build_nc():
    import concourse.bacc as bacc
    import concourse.bass as bass
    import concourse.mybir as mybir
    import concourse.tile as tile

    f32 = mybir.dt.float32
    i32 = mybir.dt.int32
    AF = mybir.ActivationFunctionType
    OP = mybir.AluOpType
    AX = mybir.AxisListType

    nc = bacc.Bacc("TRN2", target_bir_lowering=False, debug=False,
                   num_devices=NCORES)

    obs = nc.dram_tensor("obs", [BL, D + 1], f32, kind="ExternalInput")
    w1b = nc.dram_tensor("w1b", [D + 1, AD], f32, kind="ExternalInput")
    w2b = nc.dram_tensor("w2b", [AD + 1, AD], f32, kind="ExternalInput")
    id128 = nc.dram_tensor("id128", [128, 128], f32, kind="ExternalInput")
    rel = nc.dram_tensor("rel", [N_REL, D], f32, kind="ExternalInput")
    ent = nc.dram_tensor("ent", [N_ENT, D], f32, kind="ExternalInput")
    trip = nc.dram_tensor("trip", [N_TRIP, D], f32, kind="ExternalInput")
    idxr = nc.dram_tensor("idxr", [GROUPS, 128, SLOTS], i32, kind="ExternalInput")
    idxe = nc.dram_tensor("idxe", [GROUPS, 128, SLOTS], i32, kind="ExternalInput")
    idxt = nc.dram_tensor("idxt", [GROUPS, 128, SLOTS], i32, kind="ExternalInput")
    mask = nc.dram_tensor("mask", [BL, A], i32, kind="ExternalInput")
    dist = nc.dram_tensor("dist", [BL, A], f32, kind="ExternalOutput")
    entout = nc.dram_tensor("entout", [BL, 1], f32, kind="ExternalOutput")

    NW = [(0, 512), (512, 88)]          # AD=600 split into psum banks

    with tile.TileContext(nc) as tc:
        from contextlib import ExitStack
        with ExitStack() as stk:
            wp = stk.enter_context(tc.tile_pool(name="wp", bufs=1))
            mp = stk.enter_context(tc.tile_pool(name="mp", bufs=1))
            pp = stk.enter_context(tc.tile_pool(name="pp", bufs=2, space="PSUM"))

            idt = wp.tile([128, 128], f32)
            nc.sync.dma_start(out=idt[:], in_=id128[:])
            w1c1 = wp.tile([128, AD], f32, tag="w1c1")
            nc.sync.dma_start(out=w1c1[:], in_=w1b[0:128, :])
            w1c2 = wp.tile([73, AD], f32, tag="w1c2")
            nc.sync.dma_start(out=w1c2[:], in_=w1b[128:201, :])
            w2c = []
            for k in range(4):
                t = wp.tile([128, AD], f32, tag=f"w2c{k}")
                nc.sync.dma_start(out=t[:], in_=w2b[128 * k:128 * (k + 1), :])
                w2c.append(t)
            w2c4 = wp.tile([89, AD], f32, tag="w2c4")
            nc.sync.dma_start(out=w2c4[:], in_=w2b[512:601, :])

            obs_t = mp.tile([BL, D + 1], f32)
            nc.sync.dma_start(out=obs_t[:], in_=obs[:])

            # ---- obs^T (ones column of obs becomes the b1 row) ----
            obsT1 = mp.tile([128, BL], f32)
            obsT2 = mp.tile([73, BL], f32)
            pt = pp.tile([128, BL], f32, tag="tp")
            nc.tensor.transpose(pt[:], obs_t[:, 0:128], idt[0:BL, 0:BL])
            nc.vector.tensor_copy(obsT1[:], pt[:])
            pt2 = pp.tile([128, BL], f32, tag="tp")
            nc.tensor.transpose(pt2[0:73, :], obs_t[:, 128:201], idt[0:BL, 0:BL])
            nc.vector.tensor_copy(obsT2[:], pt2[0:73, :])

            # ---- H1 = relu(obs @ W1 + b1), plus a ones column for b2 ----
            h1 = mp.tile([BL, AD + 1], f32)
            nc.vector.memset(h1[:, AD:AD + 1], 1.0)
            for n0, nw in NW:
                ph = pp.tile([BL, 512], f32, tag="mm")
                nc.tensor.matmul(ph[:, 0:nw], obsT1[:], w1c1[:, n0:n0 + nw],
                                 start=True, stop=False)
                nc.tensor.matmul(ph[:, 0:nw], obsT2[:], w1c2[:, n0:n0 + nw],
                                 start=False, stop=True)
                nc.scalar.activation(h1[:, n0:n0 + nw], ph[:, 0:nw], AF.Relu)

            # ---- H1^T (with ones row for b2) ----
            h1T = []
            for k in range(4):
                t = mp.tile([128, BL], f32, tag=f"h1T{k}")
                pt3 = pp.tile([128, BL], f32, tag="tp")
                nc.tensor.transpose(pt3[:], h1[:, 128 * k:128 * (k + 1)],
                                    idt[0:BL, 0:BL])
                nc.vector.tensor_copy(t[:], pt3[:])
                h1T.append(t)
            h1T4 = mp.tile([89, BL], f32, tag="h1T4")
            pt4 = pp.tile([128, BL], f32, tag="tp")
            nc.tensor.transpose(pt4[0:89, :], h1[:, 512:601], idt[0:BL, 0:BL])
            nc.vector.tensor_copy(h1T4[:], pt4[0:89, :])

            # ---- X2 = H1 @ W2 + b2 ----
            x2 = mp.tile([BL, AD], f32)
            for n0, nw in NW:
                px = pp.tile([BL, 512], f32, tag="mm")
                for k in range(4):
                    nc.tensor.matmul(px[:, 0:nw], h1T[k][:],
                                     w2c[k][:, n0:n0 + nw],
                                     start=(k == 0), stop=False)
                nc.tensor.matmul(px[:, 0:nw], h1T4[:], w2c4[:, n0:n0 + nw],
                                 start=False, stop=True)
                nc.scalar.activation(x2[:, n0:n0 + nw], px[:, 0:nw], AF.Copy)

            # logits columns: Lcol[p, b*4+i] = logit(b, i*128+p)
            lcol = mp.tile([128, BL * 4], f32)

            bcp = stk.enter_context(tc.tile_pool(name="bcp", bufs=3, space="PSUM"))
            gp = stk.enter_context(tc.tile_pool(name="gp", bufs=2))
            xbp = stk.enter_context(tc.tile_pool(name="xbp", bufs=2))
            ixp = stk.enter_context(tc.tile_pool(name="ixp", bufs=2))
            scp = stk.enter_context(tc.tile_pool(name="scp", bufs=3))
            sgp = stk.enter_context(tc.tile_pool(name="sgp", bufs=3))

            nd = 0
            for g in range(GROUPS):
                # X2 broadcast tiles for this group's 4 batch rows
                x2bc = xbp.tile([128, BPG, AD], f32, tag="x2bc")
                for j in range(BPG):
                    lb = g * BPG + j
                    sel = idt[0:BL, lb:lb + 1].to_broadcast((BL, 128))
                    for n0, nw in NW:
                        pb = bcp.tile([128, 512], f32, tag="bc")
                        nc.tensor.matmul(pb[:, 0:nw], sel,
                                         x2[:, n0:n0 + nw],
                                         start=True, stop=True)
                        nc.scalar.activation(x2bc[:, j, n0:n0 + nw],
                                             pb[:, 0:nw], AF.Copy)

                gt = gp.tile([128, SLOTS, AD], f32, tag="g")
                for tbl, ix_dram, dd in ((rel, idxr, 0), (ent, idxe, 1),
                                         (trip, idxt, 2)):
                    ix = ixp.tile([128, SLOTS], i32, tag="ix")
                    nc.sync.dma_start(out=ix[:], in_=ix_dram[g])
                    # HW indirect DMA consumes one offset per partition:
                    # issue one gather per slot (128 rows each).
                    for s in range(SLOTS):
                        nc.gpsimd.indirect_dma_start(
                            out=gt[:, s, dd * D:(dd + 1) * D],
                            out_offset=None,
                            in_=tbl[:],
                            in_offset=bass.IndirectOffsetOnAxis(
                                ap=ix[:, s:s + 1], axis=0),
                        )

                for j in range(BPG):
                    lb = g * BPG + j
                    for i in range(4):
                        s = j * 4 + i
                        col = lb * 4 + i
                        # DVE multiplies, ACT reduces (TensorScalarPtr /
                        # tensor_tensor_reduce is rejected by this runtime)
                        scr = scp.tile([128, AD], f32, tag="sc")
                        nc.vector.tensor_tensor(
                            out=scr[:], in0=gt[:, s, :],
                            in1=x2bc[:, j, :], op=OP.mult)
                        sco = sgp.tile([128, AD], f32, tag="sg")
                        nc.scalar.activation(
                            sco[:], scr[:], AF.Copy,
                            accum_out=lcol[:, col:col + 1])
                        nd += 1

            # ---- transpose Lcol -> [b*4+i, p] then shuffle to [32, 512] ----
            lcT = mp.tile([128, 128], f32)
            plt = pp.tile([128, 512], f32, tag="mm")
            nc.tensor.transpose(plt[:, 0:128], lcol[:], idt[:])
            nc.scalar.activation(lcT[:], plt[:, 0:128], AF.Copy)

            lrow = mp.tile([BL, A], f32)
            nc.sync.dma_start(out=lrow[:], in_=lcT[:])

            # ---- mask ----
            mi = mp.tile([BL, A], i32)
            nc.sync.dma_start(out=mi[:], in_=mask[:])
            mf = mp.tile([BL, A], f32)
            nc.vector.tensor_copy(mf[:], mi[:])
            nc.vector.tensor_scalar_add(mf[:], mf[:], -1.0)
            nc.vector.tensor_scalar_mul(mf[:], mf[:], HUGE)
            nc.vector.tensor_add(lrow[:], lrow[:], mf[:])

            # ---- softmax + entropy ----
            negmax = mp.tile([BL, 1], f32)
            nc.vector.tensor_reduce(negmax[:], lrow[:], AX.X, OP.max,
                                    negate=True)
            ex = mp.tile([BL, A], f32)
            zsum = mp.tile([BL, 1], f32)
            nc.scalar.activation(ex[:], lrow[:], AF.Exp, bias=negmax[:],
                                 scale=1.0, accum_out=zsum[:])
            rz = mp.tile([BL, 1], f32)
            nc.vector.reciprocal(rz[:], zsum[:])
            pr = mp.tile([BL, A], f32)
            nc.vector.tensor_scalar_mul(pr[:], ex[:], rz[:])
            epsb = mp.tile([BL, 1], f32)
            nc.vector.memset(epsb[:], EPS)
            lnp = mp.tile([BL, A], f32)
            nc.scalar.activation(lnp[:], pr[:], AF.Ln, bias=epsb[:], scale=1.0)
            escr = mp.tile([BL, A], f32)
            nc.vector.tensor_tensor(out=escr[:], in0=pr[:], in1=lnp[:],
                                    op=OP.mult)
            escr2 = mp.tile([BL, A], f32)
            entc = mp.tile([BL, 1], f32)
            # Copy(-1 * x) accumulated = -sum(p * ln p)
            nc.scalar.activation(escr2[:], escr[:], AF.Copy, scale=-1.0,
                                 accum_out=entc[:])

            nc.sync.dma_start(out=dist[:], in_=pr[:])
            nc.sync.dma_start(out=entout[:], in_=entc[:])

    nc.compile()
    return nc


def _make_runner(nc):
    """Build a reusable jitted SPMD runner (mirrors bass2jax.run_bass_via_pjrt
    but caches the jitted callable so repeat calls don't re-trace)."""
    import jax
    import numpy as _np
    import concourse.mybir as mybir
    from jax.sharding import Mesh, PartitionSpec, NamedSharding
    from jax.experimental.shard_map import shard_map
    from concourse import bass2jax

    bass2jax.install_neuronx_cc_hook()

    partition_name = (nc.partition_id_tensor.name
                      if nc.partition_id_tensor else None)
    in_names, out_names, out_avals, zero_shapes = [], [], [], []
    for alloc in nc.m.functions[0].allocations:
        if not isinstance(alloc, mybir.MemoryLocationSet):
            continue
        name = alloc.memorylocations[0].name
        if alloc.kind == "ExternalInput":
            if name != partition_name:
                in_names.append(name)
        elif alloc.kind == "ExternalOutput":
            shape = tuple(alloc.tensor_shape)
            dtype = mybir.dt.np(alloc.dtype)
            out_names.append(name)
            out_avals.append(jax.core.ShapedArray(shape, dtype))
            zero_shapes.append((shape, dtype))
    n_params = len(in_names)
    all_names = in_names + out_names
    if partition_name is not None:
        all_names = all_names + [partition_name]

    def _body(*args):
        operands = list(args)
        if partition_name is not None:
            operands.append(bass2jax.partition_id_tensor())
        outs = bass2jax._bass_exec_p.bind(
            *operands,
            out_avals=tuple(out_avals),
            in_names=tuple(all_names),
            out_names=tuple(out_names),
            lowering_input_output_aliases=(),
            sim_require_finite=True,
            sim_require_nnan=True,
            nc=nc,
        )
        return tuple(outs)

    devices = jax.devices()[:NCORES]
    mesh = Mesh(_np.asarray(devices), ("core",))
    n_outs = len(out_names)
    donate = tuple(range(n_params, n_params + n_outs))
    sharded = jax.jit(
        shard_map(_body, mesh=mesh,
                  in_specs=(PartitionSpec("core"),) * (n_params + n_outs),
                  out_specs=(PartitionSpec("core"),) * n_outs,
                  check_rep=False),
        donate_argnums=donate, keep_unused=True)

    def prepare(in_maps):
        """Concat per-core inputs on axis 0 and move them to the devices."""
        sh = NamedSharding(mesh, PartitionSpec("core"))
        args = []
        for name in in_names:
            cat = _np.concatenate([_np.asarray(m[name]) for m in in_maps], 0)
            args.append(jax.device_put(cat, sh))
        return args

    def fresh_outs():
        return [_np.zeros((NCORES * s[0], *s[1:]), d) for s, d in zero_shapes]

    def run(dev_args):
        outs = sharded(*dev_args, *fresh_outs())
        outs = [_np.asarray(o) for o in outs]
        return [
            {name: outs[i].reshape(NCORES, *out_avals[i].shape)[c]
             for i, name in enumerate(out_names)}
            for c in range(NCORES)
        ]

    return prepare, run, sharded


def _host_prep(inputs):
    f32 = np.float32
    obs = np.asarray(inputs["obs"], f32)
    W1 = np.asarray(inputs["W1"], f32)
    b1 = np.asarray(inputs["b1"], f32)
    W2 = np.asarray(inputs["W2"], f32)
    b2 = np.asarray(inputs["b2"], f32)
    rel = np.ascontiguousarray(np.asarray(inputs["rel_emb"], f32))
    ent = np.ascontiguousarray(np.asarray(inputs["ent_emb"], f32))
    trip = np.ascontiguousarray(np.asarray(inputs["trip_emb"], f32))
    r_space = np.asarray(inputs["r_space"], np.int32)
    e_space = np.asarray(inputs["e_space"], np.int32)
    t_id = np.asarray(inputs["triple_id"], np.int32)
    amask = np.asarray(inputs["action_mask"], np.int32)

    w1b = np.ascontiguousarray(np.concatenate([W1, b1[None, :]], 0))
    w2b = np.ascontiguousarray(np.concatenate([W2, b2[None, :]], 0))
    id128 = np.eye(128, dtype=f32)

    def mk_idx(I, c):
        # [32, 512] -> [8 groups, 128 partitions, 16 slots]
        sl = I[BL * c:BL * (c + 1)].reshape(BL, 4, 128)        # [b, i, p]
        tmp = sl.reshape(GROUPS, BPG, 4, 128)                  # [g, jb, i, p]
        return np.ascontiguousarray(tmp.transpose(0, 3, 1, 2)  # [g, p, jb, i]
                                    .reshape(GROUPS, 128, SLOTS))

    in_maps = []
    for c in range(NCORES):
        in_maps.append({
            "obs": np.ascontiguousarray(np.concatenate(
                [obs[BL * c:BL * (c + 1)],
                 np.ones((BL, 1), f32)], axis=1)),
            "w1b": w1b, "w2b": w2b, "id128": id128,
            "rel": rel, "ent": ent, "trip": trip,
            "idxr": mk_idx(r_space, c),
            "idxe": mk_idx(e_space, c),
            "idxt": mk_idx(t_id, c),
            "mask": np.ascontiguousarray(amask[BL * c:BL * (c + 1)]),
        })
    return in_maps


def _get_state():
    if "state" not in _CACHE:
        nc = _build_nc()
        prepare, run, sharded = _make_runner(nc)
        _CACHE["state"] = (nc, prepare, run, sharded)
    return _CACHE["state"]


def kernel(**inputs):
    nc, prepare, run, _ = _get_state()
    in_maps = _host_prep(inputs)
    dev_args = prepare(in_maps)
    results = run(dev_args)
    dist = np.concatenate([results[c]["dist"] for c in range(NCORES)], 0)
    entv = np.concatenate([results[c]["entout"] for c in range(NCORES)], 0)[:, 0]
    return dist.astype(np.float32), np.ascontiguousarray(entv, np.float32)
